# revision 71
# baseline (speedup 1.0000x reference)
"""Trainium2 Bass kernel for nn_AttentionPointnet (gnn_message_passing).

Data-parallel over batch: 8 NeuronCores x 1 sample each (B=8, T=4096).
Per-core program (v4 — restructured around the measured bottlenecks:
DVE top-k scan wall in phase A, DMA gather occupancy in phase B):
  - KNN: m = 2 p.p_s - |p_s|^2 on PE with f32r operands (1 cyc/row at
    512-wide vs 4 for f32; ~1e-4 abs err only perturbs rank-20 boundary
    ties). Stage-1 top-8 per 256-wide chunk (exact selection); the d2
    PSUM chunks bounce through SBUF on the otherwise-idle ACT engine
    (DVE PSUM reads pay ~65ns/op extra). 128 candidates -> rank-20
    threshold via max8/match_replace merges -> threshold mask ->
    exclusive-prefix ranks via one 128-wide bf16 triangular matmul ->
    GPSIMD local_scatter compacts indices AND the candidate d2 values,
    so dis = sqrt(|p_t|^2 - m) comes straight from the selection pass
    and the baseline's separate p-row gather is gone. The per-tile
    prefix/compaction stage is software-pipelined one tile behind the
    scans so its PE ops never block the next tile's d2 matmuls in PE's
    in-order queue.
  - The per-neighbor score payload G[s,i] = p_s . wc_i[1:4] rides in the
    block-0 gather: netdram0 is a host-precomputed [T, 256] bf16 table
    [net0 | G | pad] (512B rows cost the same per descriptor as 256B
    ones given the <512B latency doubling), so block-0 pooling, scores,
    and softmax for ALL blocks come from one gather, and block 0's whole
    attention pass + half its resnet run inside phase A under the DVE
    scan wall.
  - Softmax per 2-tile gather batch: fused (dis*wc0 + G) via
    scalar_tensor_tensor reading G directly out of the gather slab,
    exp on ACT in bf16, per-block weights into w20all.
  - Weighted K-sum on PE: dstack diag(w_k) built on DVE (bf16 2x), 20
    matmuls accumulate att_raw^T per tile (lhsT fp8 x rhs bf16 mixed for
    blocks 1-5; matmul cost keys on the moving operand's dtype).
  - Blocks 1-5: net rows stored in DRAM as fp8 e4m3 at 256B stride and
    gathered with a hand-emitted DMAGatherAnt (128B payload per
    descriptor at stride_bytes_256=1 — the bass wrapper's
    elem_size%256B assert is stride-only in the descriptor format),
    halving gather DMA occupancy vs bf16; validated on hardware.
    Gathers batched 4 tiles per call (994ns desc-gen amortized), 4 slab
    buffers deep so the gather chain hides under attention compute.
    Resnet matmuls bf16 512-wide; residual in f32 via DVE. Row-major
    net rebuilt per block with DMA xbar transposes on the idle SP queue
    (14ns/16x128-tile) + an fp8 downcast on ACT; one netdram write per
    512-row group. Final projection fused into block 5's group loop.
  - Output written feature-major [CDIM, T]; host transposes.
"""

import sys

for _p in ("/opt/trn_rl_repo", "/root/.axon_site/_ro/trn_rl_repo"):
    if _p not in sys.path:
        sys.path.append(_p)

import numpy as np
import ml_dtypes

import concourse.bass as bass
import concourse.bacc as bacc
import concourse.mybir as mybir
import concourse.tile as tile
from concourse import library_config

F32 = mybir.dt.float32
F32R = mybir.dt.float32r
U16 = mybir.dt.uint16
I16 = mybir.dt.int16
BF16 = mybir.dt.bfloat16
AF = mybir.ActivationFunctionType
ALU = mybir.AluOpType
AX = mybir.AxisListType

F8 = mybir.dt.float8e4

B, T, D, H, NB, K, CDIM = 8, 4096, 3, 128, 6, 20, 128
NT = T // 128      # 32 t-tiles
NCH = 16           # 256-wide chunks for stage-1 max8 (keeps selection exact)
NCAND = NCH * 8    # 128 candidates
CHW = T // NCH     # chunk width (256)
HALF = NT // 2
ND0W = 256         # netdram0 row: [net0 bf16 128 | G bf16 6 | pad]
ND8W = 256         # fp8 netdram row stride in bytes (payload 128)


def build_program():
    nc = bacc.Bacc("TRN2", target_bir_lowering=False, debug=False)

    # ---- DRAM I/O ----
    d_lhsT4 = nc.dram_tensor("lhsT4", [4, T], F32R, kind="ExternalInput")
    d_rhs4 = nc.dram_tensor("rhs4", [4, T], F32R, kind="ExternalInput")
    d_wpos4 = nc.dram_tensor("wpos4", [4, H], F32R, kind="ExternalInput")
    d_sqt = nc.dram_tensor("sqt", [128, NT], F32, kind="ExternalInput")
    d_nd0 = nc.dram_tensor("nd0", [T, ND0W], BF16, kind="ExternalInput")
    d_wc0 = nc.dram_tensor("wc0c", [128, NB], F32, kind="ExternalInput")
    d_b0col = nc.dram_tensor("b0col", [128, NB], F32, kind="ExternalInput")
    d_w0a = nc.dram_tensor("w0a", [NB, H, H], BF16, kind="ExternalInput")
    d_w0b = nc.dram_tensor("w0b", [NB, H, H], BF16, kind="ExternalInput")
    d_w1 = nc.dram_tensor("w1", [NB, H, H], BF16, kind="ExternalInput")
    d_wsa = nc.dram_tensor("wsa", [NB, H, H], BF16, kind="ExternalInput")
    d_wsb = nc.dram_tensor("wsb", [NB, H, H], BF16, kind="ExternalInput")
    d_wo = nc.dram_tensor("wo", [NB, H, H], BF16, kind="ExternalInput")
    d_wcf = nc.dram_tensor("wcfbf", [H, CDIM], BF16, kind="ExternalInput")
    d_identbf = nc.dram_tensor("identbf", [128, 128], BF16, kind="ExternalInput")
    d_idrep = nc.dram_tensor("identrep", [128, 128 * K], BF16, kind="ExternalInput")
    d_ltbf = nc.dram_tensor("ltbf", [128, 128], BF16, kind="ExternalInput")
    d_browbf = nc.dram_tensor("browbf", [1, 18 * 128], BF16, kind="ExternalInput")
    d_coff = nc.dram_tensor("chunkoff", [128, NCAND], U16, kind="ExternalInput")
    d_repm = nc.dram_tensor("repmat", [16, 128], F32, kind="ExternalInput")
    d_out = nc.dram_tensor("outp", [CDIM, T], F32, kind="ExternalOutput")

    from contextlib import ExitStack

    with tile.TileContext(nc) as tc:
        with ExitStack() as stk:
            constp = stk.enter_context(tc.tile_pool(name="const", bufs=1))
            pers = stk.enter_context(tc.tile_pool(name="pers", bufs=1))
            dramp = stk.enter_context(tc.tile_pool(name="dram", bufs=2, space="DRAM"))
            gdramp = stk.enter_context(tc.tile_pool(name="gdram", bufs=NT, space="DRAM"))
            smallp = stk.enter_context(tc.tile_pool(name="small", bufs=3))
            dstkp = stk.enter_context(tc.tile_pool(name="dstk", bufs=2))
            gbufp = stk.enter_context(tc.tile_pool(name="gbuf", bufs=2))
            sbw = stk.enter_context(tc.tile_pool(name="sbw", bufs=3))
            outbp = stk.enter_context(tc.tile_pool(name="outb", bufs=1))
            psatt = stk.enter_context(tc.tile_pool(name="psatt", bufs=2, space="PSUM"))
            psbig = stk.enter_context(tc.tile_pool(name="psbig", bufs=2, space="PSUM"))
            stkA = ExitStack()
            slabp = stkA.enter_context(tc.tile_pool(name="slabA", bufs=2))
            persA = stkA.enter_context(tc.tile_pool(name="persA", bufs=1))
            mcpy = stkA.enter_context(tc.tile_pool(name="mcpy", bufs=2))
            constA = stkA.enter_context(tc.tile_pool(name="constA", bufs=1))
            psmarr = stkA.enter_context(tc.tile_pool(name="psmarr", bufs=3, space="PSUM"))
            psbf = stkA.enter_context(tc.tile_pool(name="psbf", bufs=1, space="PSUM"))

            nc.gpsimd.load_library(library_config.local_scatter)

            # ---- constants ----
            rhs4 = constA.tile([4, T], F32R, tag="rhs4")
            lhsT4 = constA.tile([4, T], F32R, tag="lhsT4")
            wpos4 = constA.tile([4, H], F32R, tag="wpos4")
            sqt = constA.tile([128, NT], F32, tag="sqt")
            wc0c = constA.tile([128, NB], F32, tag="wc0c")
            b0col = constp.tile([128, NB], F32, tag="b0col")
            identbf = constA.tile([128, 128], BF16, tag="identbf")
            idrep = constp.tile([128, 128 * K], BF16, tag="idrep")
            ltbf = constA.tile([128, 128], BF16, tag="ltbf")
            browbf = constp.tile([1, 18 * 128], BF16, tag="browbf")
            coff = constA.tile([128, NCAND], U16, tag="coff")
            repm = constA.tile([16, 128], F32, tag="repm")
            w0a = constp.tile([128, NB, H], BF16, tag="w0a")
            w0b = constp.tile([128, NB, H], BF16, tag="w0b")
            w1 = constp.tile([128, NB, H], BF16, tag="w1")
            wsa = constp.tile([128, NB, H], BF16, tag="wsa")
            wsb = constp.tile([128, NB, H], BF16, tag="wsb")
            wo = constp.tile([128, NB, H], BF16, tag="wo")
            wcfbf = constp.tile([128, CDIM], BF16, tag="wcfbf")

            nc.sync.dma_start(rhs4[:, :], d_rhs4.ap())
            nc.sync.dma_start(lhsT4[:, :], d_lhsT4.ap())
            nc.sync.dma_start(wpos4[:, :], d_wpos4.ap())
            nc.sync.dma_start(sqt[:, :], d_sqt.ap())
            nc.sync.dma_start(wc0c[:, :], d_wc0.ap())
            nc.sync.dma_start(b0col[:, :], d_b0col.ap())
            nc.sync.dma_start(identbf[:, :], d_identbf.ap())
            nc.sync.dma_start(idrep[:, :], d_idrep.ap())
            nc.sync.dma_start(ltbf[:, :], d_ltbf.ap())
            nc.sync.dma_start(browbf[:, :], d_browbf.ap())
            nc.sync.dma_start(coff[:, :], d_coff.ap())
            nc.sync.dma_start(repm[:, :], d_repm.ap())
            nc.sync.dma_start(wcfbf[:, :], d_wcf.ap())
            for sb_t, dr in (
                (w0a, d_w0a), (w0b, d_w0b), (w1, d_w1),
                (wsa, d_wsa), (wsb, d_wsb), (wo, d_wo),
            ):
                nc.sync.dma_start(
                    sb_t[:, :, :], dr.ap().rearrange("i hin hout -> hin i hout")
                )

            # ---- persistent activations ----
            netA = pers.tile([128, T], F32, tag="netA")
            netB = pers.tile([128, T], F32, tag="netB")
            idx16 = pers.tile([128, NT * 160], I16, tag="idx16")
            w20all = pers.tile([128, NB * NT * K], BF16, tag="w20all")
            dis = persA.tile([128, NT * K], BF16, tag="dis")
            att0T = persA.tile([128, T], BF16, tag="att0T")   # block-0 att (pre-relu)
            relu0 = persA.tile([128, T], BF16, tag="relu0")   # block-0 relu(att)

            # net0 feature-major: netA = wpos4.T @ [p;1]  (f32r, 512-wide)
            for c in range(8):
                psn = psmarr.tile([128, 512], F32, tag="psmarr")
                nc.tensor.matmul(psn[:, :], lhsT=wpos4[:, :],
                                 rhs=lhsT4[:, c * 512:(c + 1) * 512],
                                 start=True, stop=True)
                nc.scalar.copy(netA[:, c * 512:(c + 1) * 512], psn[:, :])

            idrep3 = idrep[:, :].rearrange("p (s k) -> p s k", k=K)

            def attention(i, j, lhs_tile, dst_relu, dst_att, att_on_dve=False,
                          psApool=None, dstack3=None):
                """One tile's weighted K-sum + output proj for block i.
                lhs_tile(k) -> [128, H] bf16 AP for pooled row k."""
                if dstack3 is None:
                    dstack = dstkp.tile([128, 128 * K], BF16, tag="dstack")
                    dstack3 = dstack[:, :].rearrange("p (s k) -> p s k", k=K)
                    wbase = w20all[:, (i * NT + j) * K:(i * NT + j + 1) * K]
                    wb = bass.AP(tensor=wbase.tensor, offset=wbase.offset,
                                 ap=[wbase.ap[0], [0, 128], [1, K]])
                    nc.vector.tensor_tensor(dstack3, idrep3, wb, op=ALU.mult)
                psR = psatt.tile([128, 128], F32, tag="psatt")
                for k in range(K):
                    nc.tensor.matmul(
                        psR[:, :], lhsT=lhs_tile(k), rhs=dstack3[:, :, k],
                        start=(k == 0), stop=(k == K - 1),
                    )
                attrawT = sbw.tile([128, 128], BF16, tag="attrawT")
                nc.scalar.copy(attrawT[:, :], psR[:, :])
                if psApool is not None:
                    psA = psApool.tile([128, 128], F32, tag="psA")
                else:
                    psA = psatt.tile([128, 128], F32, tag="psatt")
                nc.tensor.matmul(psA[:, :], lhsT=wo[:, i, :], rhs=attrawT[:, :],
                                 start=True, stop=False)
                nc.tensor.matmul(psA[:, :],
                                 lhsT=browbf[:, (6 + i) * 128:(7 + i) * 128],
                                 rhs=browbf[:, 13 * 128:13 * 128 + 128], start=False, stop=True)
                nc.scalar.activation(dst_relu, psA[:, :], AF.Relu)
                if att_on_dve:
                    nc.vector.tensor_copy(dst_att, psA[:, :])
                else:
                    nc.scalar.copy(dst_att, psA[:, :])

            netdram = dramp.tile([T, ND8W], F8, tag="netdram")

            def b0_group(g):
                nsl = slice(g * 512, (g + 1) * 512)
                reluN = gbufp.tile([128, 512], BF16, tag="reluN")
                nc.scalar.activation(reluN[:, :], netA[:, nsl], AF.Relu)
                netbf = gbufp.tile([128, 512], BF16, tag="netbf")
                nc.scalar.copy(netbf[:, :], netA[:, nsl])
                ps1 = psbig.tile([128, 512], F32, tag="psbig")
                nc.tensor.matmul(ps1[:, :], lhsT=w0a[:, 0, :], rhs=reluN[:, :],
                                 start=True, stop=False)
                nc.tensor.matmul(ps1[:, :], lhsT=w0b[:, 0, :],
                                 rhs=relu0[:, nsl], start=False, stop=True)
                hrelu = gbufp.tile([128, 512], BF16, tag="hrelu")
                nc.scalar.activation(hrelu[:, :], ps1[:, :], AF.Relu,
                                     bias=b0col[:, 0:1], scale=1.0)
                ps2 = psbig.tile([128, 512], F32, tag="psbig")
                nc.tensor.matmul(ps2[:, :], lhsT=w1[:, 0, :], rhs=hrelu[:, :],
                                 start=True, stop=False)
                nc.tensor.matmul(ps2[:, :], lhsT=wsa[:, 0, :], rhs=netbf[:, :],
                                 start=False, stop=False)
                nc.tensor.matmul(ps2[:, :], lhsT=wsb[:, 0, :],
                                 rhs=att0T[:, nsl], start=False, stop=False)
                nc.tensor.matmul(ps2[:, :], lhsT=browbf[:, 0:128],
                                 rhs=browbf[:, 13 * 128:13 * 128 + 512], start=False, stop=True)
                nc.vector.tensor_copy(netB[:, nsl], ps2[:, :])
                # row-major bf16 of the new net for block-1 gathers
                nob = gbufp.tile([128, 512], BF16, tag="nob")
                nc.scalar.copy(nob[:, :], ps2[:, :])
                nrow4 = sbw.tile([128, 512], BF16, tag="nrow4")
                for jj in range(4):
                    nc.sync.dma_start_transpose(
                        nrow4[:, jj * 128:(jj + 1) * 128],
                        nob[:, jj * 128:(jj + 1) * 128])
                nrowf8 = sbw.tile([128, 512], F8, tag="nrowf8")
                nc.scalar.copy(nrowf8[:, :], nrow4[:, :])
                nc.sync.dma_start(
                    netdram[g * 512:(g + 1) * 512, 0:H].rearrange(
                        "(jj t) h -> t jj h", jj=4),
                    nrowf8[:, :].rearrange("t (jj h) -> t jj h", h=H))


            # ================= Phase A: KNN + block-0 attention =============
            for h in range(2):
                if h > 0:
                    nc.gpsimd.load_library(library_config.local_scatter)
                tiles = list(range(h * HALF, (h + 1) * HALF))
                def scan_stage(j):
                    # d2 row (negated, up to t-const): m = 2 p.p_s - |p_s|^2.
                    # Stage-1 max8/max_index read each 512-chunk straight from
                    # PSUM (no SBUF copy of the distance row at all).
                    cand = smallp.tile([128, NCAND], F32, tag="cand")
                    lidx = smallp.tile([128, NCAND], U16, tag="lidx")
                    for cc in range(NCH // 2):
                        ps = psmarr.tile([128, 512], F32, tag="psmarr")
                        nc.tensor.matmul(ps[:, :],
                                         lhsT=lhsT4[:, j * 128:(j + 1) * 128],
                                         rhs=rhs4[:, cc * 512:(cc + 1) * 512],
                                         start=True, stop=True)
                        # bounce through SBUF on the (idle) ACT engine: DVE
                        # PSUM reads pay ~65ns extra init per op, 64 ops/tile
                        mch = mcpy.tile([128, 512], F32, tag="mch")
                        nc.scalar.copy(mch[:, :], ps[:, :])
                        for ci in range(2):
                            c = cc * 2 + ci
                            pss = mch[:, ci * CHW:(ci + 1) * CHW]
                            nc.vector.max(cand[:, c * 8:(c + 1) * 8], pss)
                            nc.vector.max_index(lidx[:, c * 8:(c + 1) * 8],
                                                cand[:, c * 8:(c + 1) * 8],
                                                pss)
                    vals = smallp.tile([128, 24], F32, tag="vals")
                    wk1 = smallp.tile([128, NCAND], F32, tag="wk1")
                    nc.vector.max(vals[:, 0:8], cand[:, :])
                    nc.vector.match_replace(wk1[:, :], vals[:, 0:8], cand[:, :],
                                            -1e30)
                    nc.vector.max(vals[:, 8:16], wk1[:, :])
                    nc.vector.match_replace(wk1[:, :], vals[:, 8:16], wk1[:, :],
                                            -1e30)
                    nc.vector.max(vals[:, 16:24], wk1[:, :])
                    nc.vector.tensor_tensor(lidx[:, :], lidx[:, :], coff[:, :],
                                            op=ALU.add)
                    Om = smallp.tile([128, NCAND], BF16, tag="Om")
                    nc.vector.tensor_scalar(Om[:, :], cand[:, :], vals[:, 19:20],
                                            None, op0=ALU.is_ge)
                    candbf = smallp.tile([128, NCAND], BF16, tag="candbf")
                    nc.scalar.copy(candbf[:, :], cand[:, :])
                    return (j, cand, lidx, Om, candbf)

                def compact_stage(state):
                    # prefix ranks on PE + compaction; deferred one tile so
                    # these PE ops never sit ahead of the next tile's d2
                    # matmuls in PE's in-order queue.
                    j, cand, lidx, Om, candbf = state
                    psT0 = psbf.tile([128, 128], BF16, tag="psbf")
                    nc.tensor.transpose(psT0[:, :], Om[:, :], identbf[:, :])
                    ot0 = smallp.tile([128, 128], BF16, tag="ot0")
                    nc.scalar.copy(ot0[:, :], psT0[:, :])
                    psP0 = psatt.tile([128, 128], F32, tag="psatt")
                    nc.tensor.matmul(psP0[:, :], lhsT=ltbf[:, :],
                                     rhs=ot0[:, :], start=True, stop=True)
                    pf0 = smallp.tile([128, 128], BF16, tag="pf0")
                    nc.scalar.copy(pf0[:, :], psP0[:, :])
                    psB0 = psbf.tile([128, 128], BF16, tag="psbf")
                    nc.tensor.transpose(psB0[:, :], pf0[:, :],
                                        identbf[:, :])
                    pfx = smallp.tile([128, NCAND], BF16, tag="pfx")
                    nc.scalar.copy(pfx[:, :], psB0[:, :])
                    om1 = smallp.tile([128, NCAND], BF16, tag="om1")
                    nc.vector.tensor_scalar(om1[:, :], Om[:, :], -1.0, None,
                                            op0=ALU.add)
                    nc.vector.tensor_tensor(pfx[:, :], pfx[:, :], Om[:, :],
                                            op=ALU.mult)
                    sidx = smallp.tile([128, NCAND], I16, tag="sidx")
                    nc.vector.tensor_tensor(sidx[:, :], pfx[:, :], om1[:, :],
                                            op=ALU.add)
                    gidx = smallp.tile([128, 32], I16, tag="gidx")
                    nc.gpsimd.local_scatter(gidx[:, :], lidx[:, :], sidx[:, :],
                                            channels=128, num_elems=32,
                                            num_idxs=NCAND)
                    dval = smallp.tile([128, 32], BF16, tag="dval")
                    nc.gpsimd.local_scatter(dval[:, :], candbf[:, :], sidx[:, :],
                                            channels=128, num_elems=32,
                                            num_idxs=NCAND)
                    dsl = dis[:, j * K:(j + 1) * K]
                    nc.vector.tensor_scalar(dsl, dval[:, 0:K], -1.0,
                                            sqt[:, j:j + 1],
                                            op0=ALU.mult, op1=ALU.add)
                    nc.vector.tensor_scalar_max(dsl, dsl, 1e-12)
                    # wrapped int16 gather-index image (DRAM round-trip on the
                    # Activation DMA queue, off the SP queue)
                    gdram = gdramp.tile([128, K], I16, tag="gdram")
                    nc.scalar.dma_start(gdram[:, :], gidx[:, 0:K])
                    srca = bass.AP(
                        tensor=gdram[:, :].tensor,
                        offset=gdram[:, :].offset,
                        ap=[[K, 16], [1, K], [16 * K, 8]],
                    )
                    ixq = smallp.tile([16, 160], I16, tag="ixq")
                    nc.scalar.dma_start(
                        ixq[:, :].rearrange("q (k g) -> q k g", k=K), srca)
                    ixqf = smallp.tile([16, 160], F32, tag="ixqf")
                    nc.scalar.copy(ixqf[:, :], ixq[:, :])
                    psI = psmarr.tile([128, 512], F32, tag="psmarr")
                    nc.tensor.matmul(psI[:, 0:160], lhsT=repm[:, :],
                                     rhs=ixqf[:, :], start=True, stop=True)
                    nc.scalar.copy(idx16[:, j * 160:(j + 1) * 160],
                                   psI[:, 0:160])

                pend_scan = None
                for j in tiles:
                    st = scan_stage(j)
                    if pend_scan is not None:
                        compact_stage(pend_scan)
                    pend_scan = st
                compact_stage(pend_scan)
                # batched sqrt for this half's distances (bf16)
                hsl = dis[:, h * HALF * K:(h + 1) * HALF * K]
                nc.scalar.activation(hsl, hsl, AF.Sqrt)

                # ---- gather section: block-0 slabs, softmax, block-0 att ----
                nc.gpsimd.load_library(library_config.mlp)
                for bp in range(HALF // 2):  # 2-tile batches
                    j0 = h * HALF + bp * 2
                    slabr = slabp.tile([128, 4 * K, H], BF16, tag="slab")
                    slab0 = slabr[:, :, :].rearrange(
                        "p (j two) h -> p j (two h)", two=2)
                    nc.gpsimd.dma_gather(
                        out_ap=slab0, in_ap=d_nd0.ap(),
                        idxs_ap=idx16[:, j0 * 160:(j0 + 2) * 160],
                        num_idxs=128 * K * 2, num_idxs_reg=128 * K * 2,
                        elem_size=ND0W, single_packet=False,
                    )
                    # softmax for both tiles, all 6 blocks
                    d2sl = dis[:, j0 * K:(j0 + 2) * K]
                    sraw = slabr[:, :, :]
                    for i in range(NB):
                        # G[idx] payload: raw row 2*slot+1, col i
                        gap = bass.AP(
                            tensor=sraw.tensor,
                            offset=sraw.offset + H + i,
                            ap=[sraw.ap[0], [2 * H, 2 * K]],
                        )
                        sc = smallp.tile([128, 2 * K], BF16, tag="sc")
                        nc.vector.scalar_tensor_tensor(
                            sc[:, :], d2sl, wc0c[:, i:i + 1], gap,
                            op0=ALU.mult, op1=ALU.add)
                        esc = smallp.tile([128, 2 * K], BF16, tag="esc")
                        nc.scalar.activation(esc[:, :], sc[:, :], AF.Exp)
                        esc3 = esc[:, :].rearrange("p (j k) -> p j k", k=K)
                        den = smallp.tile([128, 2], F32, tag="den")
                        nc.vector.tensor_reduce(den[:, :], esc3, axis=AX.X,
                                                op=ALU.add)
                        rden = smallp.tile([128, 2], F32, tag="rden")
                        nc.vector.reciprocal(rden[:, :], den[:, :])
                        rb = rden[:, :]
                        rbc = bass.AP(tensor=rb.tensor, offset=rb.offset,
                                      ap=[rb.ap[0], [1, 2], [0, K]])
                        w20s = w20all[:, i * NT * K + j0 * K:
                                      i * NT * K + (j0 + 2) * K].rearrange(
                            "p (j k) -> p j k", k=K)
                        nc.vector.tensor_tensor(w20s, esc3, rbc, op=ALU.mult)
                    # block-0 attention for both tiles
                    for jj in range(2):
                        j = j0 + jj
                        attention(
                            0, j,
                            lambda k, _jj=jj: slabr[:, 2 * (_jj * K + k), :],
                            relu0[:, j * 128:(j + 1) * 128],
                            att0T[:, j * 128:(j + 1) * 128],
                        )

                if h == 0:
                    for g in range(4):
                        b0_group(g)

            def gather128(out_ap, in_ap, idxs_ap, num_idxs):
                """fp8 gather: 128B payload per descriptor at 256B row stride.
                The bass wrapper requires elem_size_bytes % 256 == 0; the
                descriptor format only constrains the STRIDE to 256B units,
                so emit InstDMAGatherAnt directly."""
                gp = nc.gpsimd
                _in = gp.lower_ap_dma(in_ap, for_custom_bir_dma=True)
                _idx = gp.lower_ap(idxs_ap)
                _out = gp.lower_ap(out_ap)
                return gp.add_instruction(mybir.InstDMAGatherAnt(
                    name=gp.bass.get_next_instruction_name(),
                    ins=[*_in, _idx,
                         gp.lower_val_access(gp.to_reg(num_idxs))],
                    outs=[_out],
                    transpose=False,
                    num_idxs=num_idxs,
                    elem_size=H,
                    stride_bytes_256=1,
                    gen_mode=0,
                    single_packet=False,
                    queue_num=0,
                    sbuf_tokens_per_rank=0,
                    sbuf_free_dim_per_rank=0,
                    sbuf_free_dim_pad_per_rank=0,
                    sbuf_byte_offset=0,
                ))

            for g in range(4, 8):
                b0_group(g)

            # ================= blocks 1..5 =================
            stkA.close()
            slabp8 = stk.enter_context(tc.tile_pool(name="slab8", bufs=4))
            dstk2p = stk.enter_context(tc.tile_pool(name="dstk2", bufs=2))
            gbufB = stk.enter_context(tc.tile_pool(name="gbufB", bufs=3))
            psattB = stk.enter_context(tc.tile_pool(name="psattB", bufs=2, space="PSUM"))
            psbigB = stk.enter_context(tc.tile_pool(name="psbigB", bufs=2, space="PSUM"))
            for i in range(1, NB):
                nin = netB if i % 2 == 1 else netA
                nout = netA if i % 2 == 1 else netB
                if i < NB - 1:
                    netdram_next = dramp.tile([T, ND8W], F8, tag="netdram")
                else:
                    netdram_next = None
                def att_stage(g):
                    slab = slabp8.tile([128, 4 * K, H], F8, tag="slab8")
                    gather128(slab[:, :, :], netdram[:, 0:H],
                              idx16[:, (4 * g) * 160:(4 * g + 4) * 160],
                              128 * K * 4)
                    attT4 = gbufB.tile([128, 512], BF16, tag="attT4B")
                    reluA4 = gbufB.tile([128, 512], BF16, tag="reluA4B")
                    for jp in range(2):
                        # one DVE op builds BOTH tiles' diag stacks (halves
                        # the DVE->PE handoff count in the unit chain)
                        ds2 = dstk2p.tile([128, 2 * 128 * K], BF16, tag="ds2")
                        ds24 = ds2[:, :].rearrange(
                            "p (jj s k) -> p jj s k", jj=2, k=K)
                        j0 = 4 * g + 2 * jp
                        wbase = w20all[:, (i * NT + j0) * K:
                                       (i * NT + j0 + 2) * K]
                        wb = bass.AP(
                            tensor=wbase.tensor, offset=wbase.offset,
                            ap=[wbase.ap[0], [K, 2], [0, 128], [1, K]])
                        idr4 = bass.AP(tensor=idrep3.tensor,
                                       offset=idrep3.offset,
                                       ap=[idrep3.ap[0], [0, 2],
                                           idrep3.ap[1], idrep3.ap[2]])
                        nc.vector.tensor_tensor(ds24, idr4, wb, op=ALU.mult)
                        for jj2 in range(2):
                            jj = 2 * jp + jj2
                            j = 4 * g + jj
                            attention(
                                i, j,
                                lambda k, _jj=jj: slab[:, _jj * K + k, :],
                                reluA4[:, jj * 128:(jj + 1) * 128],
                                attT4[:, jj * 128:(jj + 1) * 128],
                                psApool=psattB,
                                dstack3=ds24[:, jj2, :, :],
                            )
                    return (g, attT4, reluA4)

                def resnet_stage(state):
                    # deferred one group so PE's in-order queue runs the next
                    # group's attention matmuls instead of stalling on ACT's
                    # hrelu (same fix that unblocked the phase-A scans)
                    g, attT4, reluA4 = state
                    nsl = slice(g * 512, (g + 1) * 512)
                    reluNt = gbufp.tile([128, 512], BF16, tag="reluN")
                    nc.scalar.activation(reluNt[:, :], nin[:, nsl], AF.Relu)
                    reluN = reluNt[:, :]
                    netbft = gbufp.tile([128, 512], BF16, tag="netbf")
                    nc.scalar.copy(netbft[:, :], nin[:, nsl])
                    netbf = netbft[:, :]
                    ps1 = psbig.tile([128, 512], F32, tag="psbig")
                    nc.tensor.matmul(ps1[:, :], lhsT=w0a[:, i, :],
                                     rhs=reluN, start=True, stop=False)
                    nc.tensor.matmul(ps1[:, :], lhsT=w0b[:, i, :],
                                     rhs=reluA4[:, :], start=False, stop=True)
                    hrelu = gbufp.tile([128, 512], BF16, tag="hrelu")
                    nc.scalar.activation(hrelu[:, :], ps1[:, :], AF.Relu,
                                         bias=b0col[:, i:i + 1], scale=1.0)
                    ps2 = psbigB.tile([128, 512], F32, tag="psbigB")
                    nc.tensor.matmul(ps2[:, :], lhsT=w1[:, i, :],
                                     rhs=hrelu[:, :], start=True, stop=False)
                    nc.tensor.matmul(ps2[:, :], lhsT=wsa[:, i, :],
                                     rhs=netbf, start=False, stop=False)
                    nc.tensor.matmul(ps2[:, :], lhsT=wsb[:, i, :],
                                     rhs=attT4[:, :], start=False, stop=False)
                    nc.tensor.matmul(ps2[:, :],
                                     lhsT=browbf[:, i * 128:(i + 1) * 128],
                                     rhs=browbf[:, 13 * 128:13 * 128 + 512],
                                     start=False, stop=True)
                    nc.vector.tensor_tensor(nout[:, nsl], ps2[:, :],
                                            nin[:, nsl], op=ALU.add)
                    if i == NB - 1:
                        # fused final projection for this group
                        n6bf = gbufp.tile([128, 512], BF16, tag="nob")
                        nc.scalar.copy(n6bf[:, :], nout[:, nsl])
                        psF = psbig.tile([128, 512], F32, tag="psbig")
                        nc.tensor.matmul(psF[:, :], lhsT=wcfbf[:, :],
                                         rhs=n6bf[:, :], start=True, stop=False)
                        nc.tensor.matmul(psF[:, :],
                                         lhsT=browbf[:, 12 * 128:13 * 128],
                                         rhs=browbf[:, 13 * 128:13 * 128 + 512],
                                         start=False, stop=True)
                        og = outbp.tile([128, 512], F32, tag="og")
                        nc.scalar.copy(og[:, :], psF[:, :])
                        nc.sync.dma_start(d_out.ap()[:, nsl], og[:, :])
                    if netdram_next is not None:
                        nob = gbufp.tile([128, 512], BF16, tag="nob")
                        nc.scalar.copy(nob[:, :], nout[:, nsl])
                        nrow4 = sbw.tile([128, 512], BF16, tag="nrow4")
                        for jj in range(4):
                            nc.sync.dma_start_transpose(
                                nrow4[:, jj * 128:(jj + 1) * 128],
                                nob[:, jj * 128:(jj + 1) * 128])
                        nrowf8 = sbw.tile([128, 512], F8, tag="nrowf8")
                        nc.scalar.copy(nrowf8[:, :], nrow4[:, :])
                        nc.sync.dma_start(
                            netdram_next[g * 512:(g + 1) * 512, 0:H].rearrange(
                                "(jj t) h -> t jj h", jj=4),
                            nrowf8[:, :].rearrange("t (jj h) -> t jj h", h=H))

                pend_g = None
                for g in range(8):
                    stg = att_stage(g)
                    if pend_g is not None:
                        resnet_stage(pend_g)
                    pend_g = stg
                resnet_stage(pend_g)
                netdram = netdram_next

    nc.compile()
    return nc


def make_inputs(p_all, weights):
    """Build the per-core input maps. p_all: [B, T, D] f32."""
    w = weights
    bf = ml_dtypes.bfloat16
    shared = {}
    wc = np.asarray(w["att_Wc"][:, :, 0], np.float32)  # [NB, 7]
    shared["wpos4"] = np.concatenate(
        [w["W_pos"], w["b_pos"][None, :]], axis=0
    ).astype(np.float32)  # [4, H]
    shared["wc0c"] = np.broadcast_to(wc[:, 0][None, :], (128, NB)).astype(
        np.float32).copy()
    shared["b0col"] = np.ascontiguousarray(
        np.asarray(w["blk_b0"], np.float32).T)  # [H, NB]
    shared["w0a"] = np.ascontiguousarray(w["blk_W0"][:, :H, :]).astype(bf)
    shared["w0b"] = np.ascontiguousarray(w["blk_W0"][:, H:, :]).astype(bf)
    shared["w1"] = np.ascontiguousarray(w["blk_W1"]).astype(bf)
    shared["wsa"] = np.ascontiguousarray(w["blk_Ws"][:, :H, :]).astype(bf)
    shared["wsb"] = np.ascontiguousarray(w["blk_Ws"][:, H:, :]).astype(bf)
    shared["wo"] = np.ascontiguousarray(w["att_Wo"]).astype(bf)
    shared["wcfbf"] = np.ascontiguousarray(w["W_c"]).astype(bf)
    shared["identbf"] = np.eye(128, dtype=np.float32).astype(bf)
    idr = np.zeros((128, 128, K), np.float32)
    for t in range(128):
        idr[t, t, :] = 1.0
    shared["identrep"] = idr.reshape(128, 128 * K).astype(bf)
    brow = np.zeros((1, 18 * 128), np.float32)
    for i in range(NB):
        brow[0, i * 128:(i + 1) * 128] = w["blk_b1"][i]
        brow[0, (6 + i) * 128:(7 + i) * 128] = w["att_bo"][i]
    brow[0, 12 * 128:13 * 128] = w["b_c"]
    brow[0, 13 * 128:17 * 128] = 1.0
    shared["browbf"] = brow.astype(bf)
    shared["ltbf"] = np.triu(np.ones((128, 128), np.float32), 1).T.astype(bf)
    co = np.zeros((128, NCAND), np.uint16)
    co[:, :] = (np.arange(NCAND) // 8 * CHW)[None, :]
    shared["chunkoff"] = co
    rm = np.zeros((16, 128), np.float32)
    for p in range(128):
        rm[p % 16, p] = 1.0
    shared["repmat"] = rm

    wcg = wc[:, 1:4]  # [NB, 3]
    wpos = np.asarray(w["W_pos"], np.float32)
    bpos = np.asarray(w["b_pos"], np.float32)

    in_maps = []
    for c in range(B):
        p = np.asarray(p_all[c], np.float32)  # [T, D]
        sq = (p * p).sum(-1)  # [T]
        m = dict(shared)
        lhsT4 = np.ones((4, T), np.float32)
        lhsT4[0:3, :] = p.T
        m["lhsT4"] = lhsT4
        rhs4 = np.empty((4, T), np.float32)
        rhs4[0:3, :] = 2.0 * p.T
        rhs4[3, :] = -sq
        m["rhs4"] = rhs4
        m["sqt"] = np.ascontiguousarray(sq.reshape(NT, 128).T)  # [128, NT]
        nd0 = np.zeros((T, ND0W), np.float32)
        nd0[:, 0:H] = p @ wpos + bpos          # net0 rows
        nd0[:, H:H + NB] = p @ wcg.T           # G[s, i]
        m["nd0"] = nd0.astype(bf)
        in_maps.append(m)
    return in_maps


_PROGRAM = None


def kernel(**inputs):
    global _PROGRAM
    p_all = np.asarray(inputs["p"], np.float32)
    assert p_all.shape == (B, T, D)
    in_maps = make_inputs(p_all, {k: np.asarray(v) for k, v in inputs.items()})
    if _PROGRAM is None:
        _PROGRAM = build_program()
    from concourse import bass_utils
    res = bass_utils.run_bass_kernel_spmd(
        _PROGRAM, in_maps, core_ids=list(range(B))
    )
    out = np.stack([r["outp"].T for r in res.results], axis=0)  # [B, T, CDIM]
    return np.ascontiguousarray(out).astype(np.float32)


if __name__ == "__main__":
    # smoke: build only
    nc = build_program()
    print("built ok")


# revision 72
# speedup vs baseline: 1.0782x; 1.0782x over previous
"""Trainium2 Bass kernel for nn_AttentionPointnet (gnn_message_passing).

Data-parallel over batch: 8 NeuronCores x 1 sample each (B=8, T=4096).
Per-core program (v4 — restructured around the measured bottlenecks:
DVE top-k scan wall in phase A, DMA gather occupancy in phase B):
  - KNN: m = 2 p.p_s - |p_s|^2 on PE with f32r operands (1 cyc/row at
    512-wide vs 4 for f32; ~1e-4 abs err only perturbs rank-20 boundary
    ties). Stage-1 top-8 per 256-wide chunk (exact selection); the d2
    PSUM chunks bounce through SBUF on the otherwise-idle ACT engine
    (DVE PSUM reads pay ~65ns/op extra). 128 candidates -> rank-20
    threshold via max8/match_replace merges -> threshold mask ->
    exclusive-prefix ranks via one 128-wide bf16 triangular matmul ->
    GPSIMD local_scatter compacts indices AND the candidate d2 values,
    so dis = sqrt(|p_t|^2 - m) comes straight from the selection pass
    and the baseline's separate p-row gather is gone. The per-tile
    prefix/compaction stage is software-pipelined one tile behind the
    scans so its PE ops never block the next tile's d2 matmuls in PE's
    in-order queue.
  - The per-neighbor score payload G[s,i] = p_s . wc_i[1:4] rides in the
    block-0 gather: netdram0 is a host-precomputed [T, 256] bf16 table
    [net0 | G | pad] (512B rows cost the same per descriptor as 256B
    ones given the <512B latency doubling), so block-0 pooling, scores,
    and softmax for ALL blocks come from one gather, and block 0's whole
    attention pass + half its resnet run inside phase A under the DVE
    scan wall.
  - Softmax per 2-tile gather batch: fused (dis*wc0 + G) via
    scalar_tensor_tensor reading G directly out of the gather slab,
    exp on ACT in bf16, per-block weights into w20all.
  - Weighted K-sum on PE: dstack diag(w_k) built on DVE (bf16 2x), 20
    matmuls accumulate att_raw^T per tile (lhsT fp8 x rhs bf16 mixed for
    blocks 1-5; matmul cost keys on the moving operand's dtype).
  - Blocks 1-5: net rows stored in DRAM as fp8 e4m3 at 256B stride and
    gathered with a hand-emitted DMAGatherAnt (128B payload per
    descriptor at stride_bytes_256=1 — the bass wrapper's
    elem_size%256B assert is stride-only in the descriptor format),
    halving gather DMA occupancy vs bf16; validated on hardware.
    Gathers batched 4 tiles per call (994ns desc-gen amortized), 4 slab
    buffers deep so the gather chain hides under attention compute.
    Resnet matmuls bf16 512-wide; residual in f32 via DVE. Row-major
    net rebuilt per block with DMA xbar transposes on the idle SP queue
    (14ns/16x128-tile) + an fp8 downcast on ACT; one netdram write per
    512-row group. Final projection fused into block 5's group loop.
  - Output written feature-major [CDIM, T]; host transposes.
"""

import sys

for _p in ("/opt/trn_rl_repo", "/root/.axon_site/_ro/trn_rl_repo"):
    if _p not in sys.path:
        sys.path.append(_p)

import numpy as np
import ml_dtypes

import concourse.bass as bass
import concourse.bacc as bacc
import concourse.mybir as mybir
import concourse.tile as tile
from concourse import library_config

F32 = mybir.dt.float32
F32R = mybir.dt.float32r
U16 = mybir.dt.uint16
I16 = mybir.dt.int16
BF16 = mybir.dt.bfloat16
AF = mybir.ActivationFunctionType
ALU = mybir.AluOpType
AX = mybir.AxisListType

F8 = mybir.dt.float8e4

B, T, D, H, NB, K, CDIM = 8, 4096, 3, 128, 6, 20, 128
NT = T // 128      # 32 t-tiles
NCH = 16           # 256-wide chunks for stage-1 max8 (keeps selection exact)
NCAND = NCH * 8    # 128 candidates
CHW = T // NCH     # chunk width (256)
HALF = NT // 2
ND0W = 256         # netdram0 row: [net0 bf16 128 | G bf16 6 | pad]
ND8W = 256         # fp8 netdram row stride in bytes (payload 128)


def build_program():
    nc = bacc.Bacc("TRN2", target_bir_lowering=False, debug=False)

    # ---- DRAM I/O ----
    d_lhsT4 = nc.dram_tensor("lhsT4", [4, T], F32R, kind="ExternalInput")
    d_rhs4 = nc.dram_tensor("rhs4", [4, T], F32R, kind="ExternalInput")
    d_wpos4 = nc.dram_tensor("wpos4", [4, H], F32R, kind="ExternalInput")
    d_sqt = nc.dram_tensor("sqt", [128, NT], F32, kind="ExternalInput")
    d_nd0 = nc.dram_tensor("nd0", [T, ND0W], BF16, kind="ExternalInput")
    d_wc0 = nc.dram_tensor("wc0c", [128, NB], F32, kind="ExternalInput")
    d_b0col = nc.dram_tensor("b0col", [128, NB], F32, kind="ExternalInput")
    d_w0a = nc.dram_tensor("w0a", [NB, H, H], BF16, kind="ExternalInput")
    d_w0b = nc.dram_tensor("w0b", [NB, H, H], BF16, kind="ExternalInput")
    d_w1 = nc.dram_tensor("w1", [NB, H, H], BF16, kind="ExternalInput")
    d_wsa = nc.dram_tensor("wsa", [NB, H, H], BF16, kind="ExternalInput")
    d_wsb = nc.dram_tensor("wsb", [NB, H, H], BF16, kind="ExternalInput")
    d_wo = nc.dram_tensor("wo", [NB, H, H], BF16, kind="ExternalInput")
    d_wcf = nc.dram_tensor("wcfbf", [H, CDIM], BF16, kind="ExternalInput")
    d_identbf = nc.dram_tensor("identbf", [128, 128], BF16, kind="ExternalInput")
    d_idrep = nc.dram_tensor("identrep", [128, 128 * K], BF16, kind="ExternalInput")
    d_ltbf = nc.dram_tensor("ltbf", [128, 128], BF16, kind="ExternalInput")
    d_browbf = nc.dram_tensor("browbf", [1, 18 * 128], BF16, kind="ExternalInput")
    d_coff = nc.dram_tensor("chunkoff", [128, NCAND], U16, kind="ExternalInput")
    d_repm = nc.dram_tensor("repmat", [16, 128], F32, kind="ExternalInput")
    d_out = nc.dram_tensor("outp", [CDIM, T], F32, kind="ExternalOutput")

    from contextlib import ExitStack

    with tile.TileContext(nc) as tc:
        with ExitStack() as stk:
            constp = stk.enter_context(tc.tile_pool(name="const", bufs=1))
            pers = stk.enter_context(tc.tile_pool(name="pers", bufs=1))
            dramp = stk.enter_context(tc.tile_pool(name="dram", bufs=2, space="DRAM"))
            gdramp = stk.enter_context(tc.tile_pool(name="gdram", bufs=NT, space="DRAM"))
            smallp = stk.enter_context(tc.tile_pool(name="small", bufs=3))
            dstkp = stk.enter_context(tc.tile_pool(name="dstk", bufs=2))
            gbufp = stk.enter_context(tc.tile_pool(name="gbuf", bufs=2))
            sbw = stk.enter_context(tc.tile_pool(name="sbw", bufs=3))
            outbp = stk.enter_context(tc.tile_pool(name="outb", bufs=1))
            psatt = stk.enter_context(tc.tile_pool(name="psatt", bufs=2, space="PSUM"))
            psbig = stk.enter_context(tc.tile_pool(name="psbig", bufs=2, space="PSUM"))
            stkA = ExitStack()
            slabp = stkA.enter_context(tc.tile_pool(name="slabA", bufs=2))
            persA = stkA.enter_context(tc.tile_pool(name="persA", bufs=1))
            mcpy = stkA.enter_context(tc.tile_pool(name="mcpy", bufs=2))
            constA = stkA.enter_context(tc.tile_pool(name="constA", bufs=1))
            psmarr = stkA.enter_context(tc.tile_pool(name="psmarr", bufs=3, space="PSUM"))
            psbf = stkA.enter_context(tc.tile_pool(name="psbf", bufs=1, space="PSUM"))

            nc.gpsimd.load_library(library_config.local_scatter)

            # ---- constants ----
            rhs4 = constA.tile([4, T], F32R, tag="rhs4")
            lhsT4 = constA.tile([4, T], F32R, tag="lhsT4")
            wpos4 = constA.tile([4, H], F32R, tag="wpos4")
            sqt = constA.tile([128, NT], F32, tag="sqt")
            wc0c = constA.tile([128, NB], F32, tag="wc0c")
            b0col = constp.tile([128, NB], F32, tag="b0col")
            identbf = constA.tile([128, 128], BF16, tag="identbf")
            idrep = constp.tile([128, 128 * K], BF16, tag="idrep")
            ltbf = constA.tile([128, 128], BF16, tag="ltbf")
            browbf = constp.tile([1, 18 * 128], BF16, tag="browbf")
            coff = constA.tile([128, NCAND], U16, tag="coff")
            repm = constA.tile([16, 128], F32, tag="repm")
            w0a = constp.tile([128, NB, H], BF16, tag="w0a")
            w0b = constp.tile([128, NB, H], BF16, tag="w0b")
            w1 = constp.tile([128, NB, H], BF16, tag="w1")
            wsa = constp.tile([128, NB, H], BF16, tag="wsa")
            wsb = constp.tile([128, NB, H], BF16, tag="wsb")
            wo = constp.tile([128, NB, H], BF16, tag="wo")
            wcfbf = constp.tile([128, CDIM], BF16, tag="wcfbf")

            nc.sync.dma_start(rhs4[:, :], d_rhs4.ap())
            nc.sync.dma_start(lhsT4[:, :], d_lhsT4.ap())
            nc.sync.dma_start(wpos4[:, :], d_wpos4.ap())
            nc.sync.dma_start(sqt[:, :], d_sqt.ap())
            nc.sync.dma_start(wc0c[:, :], d_wc0.ap())
            nc.sync.dma_start(b0col[:, :], d_b0col.ap())
            nc.sync.dma_start(identbf[:, :], d_identbf.ap())
            nc.sync.dma_start(idrep[:, :], d_idrep.ap())
            nc.sync.dma_start(ltbf[:, :], d_ltbf.ap())
            nc.sync.dma_start(browbf[:, :], d_browbf.ap())
            nc.sync.dma_start(coff[:, :], d_coff.ap())
            nc.sync.dma_start(repm[:, :], d_repm.ap())
            nc.sync.dma_start(wcfbf[:, :], d_wcf.ap())
            for sb_t, dr in (
                (w0a, d_w0a), (w0b, d_w0b), (w1, d_w1),
                (wsa, d_wsa), (wsb, d_wsb), (wo, d_wo),
            ):
                nc.sync.dma_start(
                    sb_t[:, :, :], dr.ap().rearrange("i hin hout -> hin i hout")
                )

            # ---- persistent activations ----
            netA = pers.tile([128, T], F32, tag="netA")
            netB = pers.tile([128, T], F32, tag="netB")
            idx16 = pers.tile([128, NT * 160], I16, tag="idx16")
            w20all = pers.tile([128, NB * NT * K], BF16, tag="w20all")
            dis = persA.tile([128, NT * K], BF16, tag="dis")
            att0T = persA.tile([128, T], BF16, tag="att0T")   # block-0 att (pre-relu)
            relu0 = persA.tile([128, T], BF16, tag="relu0")   # block-0 relu(att)

            # net0 feature-major: netA = wpos4.T @ [p;1]  (f32r, 512-wide)
            for c in range(8):
                psn = psmarr.tile([128, 512], F32, tag="psmarr")
                nc.tensor.matmul(psn[:, :], lhsT=wpos4[:, :],
                                 rhs=lhsT4[:, c * 512:(c + 1) * 512],
                                 start=True, stop=True)
                nc.scalar.copy(netA[:, c * 512:(c + 1) * 512], psn[:, :])

            idrep3 = idrep[:, :].rearrange("p (s k) -> p s k", k=K)

            def attention(i, j, lhs_tile, dst_relu, dst_att, att_on_dve=False,
                          psApool=None, dstack3=None):
                """One tile's weighted K-sum + output proj for block i.
                lhs_tile(k) -> [128, H] bf16 AP for pooled row k."""
                if dstack3 is None:
                    dstack = dstkp.tile([128, 128 * K], BF16, tag="dstack")
                    dstack3 = dstack[:, :].rearrange("p (s k) -> p s k", k=K)
                    wbase = w20all[:, (i * NT + j) * K:(i * NT + j + 1) * K]
                    wb = bass.AP(tensor=wbase.tensor, offset=wbase.offset,
                                 ap=[wbase.ap[0], [0, 128], [1, K]])
                    nc.vector.tensor_tensor(dstack3, idrep3, wb, op=ALU.mult)
                psR = psatt.tile([128, 128], F32, tag="psatt")
                for k in range(K):
                    nc.tensor.matmul(
                        psR[:, :], lhsT=lhs_tile(k), rhs=dstack3[:, :, k],
                        start=(k == 0), stop=(k == K - 1),
                    )
                attrawT = sbw.tile([128, 128], BF16, tag="attrawT")
                nc.scalar.copy(attrawT[:, :], psR[:, :])
                if psApool is not None:
                    psA = psApool.tile([128, 128], F32, tag="psA")
                else:
                    psA = psatt.tile([128, 128], F32, tag="psatt")
                nc.tensor.matmul(psA[:, :], lhsT=wo[:, i, :], rhs=attrawT[:, :],
                                 start=True, stop=False)
                nc.tensor.matmul(psA[:, :],
                                 lhsT=browbf[:, (6 + i) * 128:(7 + i) * 128],
                                 rhs=browbf[:, 13 * 128:13 * 128 + 128], start=False, stop=True)
                nc.scalar.activation(dst_relu, psA[:, :], AF.Relu)
                if att_on_dve:
                    nc.vector.tensor_copy(dst_att, psA[:, :])
                else:
                    nc.scalar.copy(dst_att, psA[:, :])

            netdram = dramp.tile([T, ND8W], F8, tag="netdram")

            def b0_group(g):
                nsl = slice(g * 512, (g + 1) * 512)
                reluN = gbufp.tile([128, 512], BF16, tag="reluN")
                nc.scalar.activation(reluN[:, :], netA[:, nsl], AF.Relu)
                netbf = gbufp.tile([128, 512], BF16, tag="netbf")
                nc.scalar.copy(netbf[:, :], netA[:, nsl])
                ps1 = psbig.tile([128, 512], F32, tag="psbig")
                nc.tensor.matmul(ps1[:, :], lhsT=w0a[:, 0, :], rhs=reluN[:, :],
                                 start=True, stop=False)
                nc.tensor.matmul(ps1[:, :], lhsT=w0b[:, 0, :],
                                 rhs=relu0[:, nsl], start=False, stop=True)
                hrelu = gbufp.tile([128, 512], BF16, tag="hrelu")
                nc.scalar.activation(hrelu[:, :], ps1[:, :], AF.Relu,
                                     bias=b0col[:, 0:1], scale=1.0)
                ps2 = psbig.tile([128, 512], F32, tag="psbig")
                nc.tensor.matmul(ps2[:, :], lhsT=w1[:, 0, :], rhs=hrelu[:, :],
                                 start=True, stop=False)
                nc.tensor.matmul(ps2[:, :], lhsT=wsa[:, 0, :], rhs=netbf[:, :],
                                 start=False, stop=False)
                nc.tensor.matmul(ps2[:, :], lhsT=wsb[:, 0, :],
                                 rhs=att0T[:, nsl], start=False, stop=False)
                nc.tensor.matmul(ps2[:, :], lhsT=browbf[:, 0:128],
                                 rhs=browbf[:, 13 * 128:13 * 128 + 512], start=False, stop=True)
                nc.vector.tensor_copy(netB[:, nsl], ps2[:, :])
                # row-major bf16 of the new net for block-1 gathers
                nob = gbufp.tile([128, 512], BF16, tag="nob")
                nc.scalar.copy(nob[:, :], ps2[:, :])
                nrow4 = sbw.tile([128, 512], BF16, tag="nrow4")
                for jj in range(4):
                    nc.sync.dma_start_transpose(
                        nrow4[:, jj * 128:(jj + 1) * 128],
                        nob[:, jj * 128:(jj + 1) * 128])
                nrowf8 = sbw.tile([128, 512], F8, tag="nrowf8")
                nc.scalar.copy(nrowf8[:, :], nrow4[:, :])
                nc.sync.dma_start(
                    netdram[g * 512:(g + 1) * 512, 0:H].rearrange(
                        "(jj t) h -> t jj h", jj=4),
                    nrowf8[:, :].rearrange("t (jj h) -> t jj h", h=H))


            # ================= Phase A: KNN + block-0 attention =============
            for h in range(2):
                if h > 0:
                    nc.gpsimd.load_library(library_config.local_scatter)
                tiles = list(range(h * HALF, (h + 1) * HALF))
                def scan_stage(j):
                    # d2 row (negated, up to t-const): m = 2 p.p_s - |p_s|^2.
                    # Stage-1 max8/max_index read each 512-chunk straight from
                    # PSUM (no SBUF copy of the distance row at all).
                    cand = smallp.tile([128, NCAND], F32, tag="cand")
                    lidx = smallp.tile([128, NCAND], U16, tag="lidx")
                    for cc in range(NCH // 2):
                        ps = psmarr.tile([128, 512], F32, tag="psmarr")
                        nc.tensor.matmul(ps[:, :],
                                         lhsT=lhsT4[:, j * 128:(j + 1) * 128],
                                         rhs=rhs4[:, cc * 512:(cc + 1) * 512],
                                         start=True, stop=True)
                        # bounce through SBUF on the (idle) ACT engine: DVE
                        # PSUM reads pay ~65ns extra init per op, 64 ops/tile
                        mch = mcpy.tile([128, 512], F32, tag="mch")
                        nc.scalar.copy(mch[:, :], ps[:, :])
                        for ci in range(2):
                            c = cc * 2 + ci
                            pss = mch[:, ci * CHW:(ci + 1) * CHW]
                            nc.vector.max(cand[:, c * 8:(c + 1) * 8], pss)
                            nc.vector.max_index(lidx[:, c * 8:(c + 1) * 8],
                                                cand[:, c * 8:(c + 1) * 8],
                                                pss)
                    vals = smallp.tile([128, 24], F32, tag="vals")
                    wk1 = smallp.tile([128, NCAND], F32, tag="wk1")
                    nc.vector.max(vals[:, 0:8], cand[:, :])
                    nc.vector.match_replace(wk1[:, :], vals[:, 0:8], cand[:, :],
                                            -1e30)
                    nc.vector.max(vals[:, 8:16], wk1[:, :])
                    nc.vector.match_replace(wk1[:, :], vals[:, 8:16], wk1[:, :],
                                            -1e30)
                    nc.vector.max(vals[:, 16:24], wk1[:, :])
                    nc.vector.tensor_tensor(lidx[:, :], lidx[:, :], coff[:, :],
                                            op=ALU.add)
                    Om = smallp.tile([128, NCAND], BF16, tag="Om")
                    nc.vector.tensor_scalar(Om[:, :], cand[:, :], vals[:, 19:20],
                                            None, op0=ALU.is_ge)
                    candbf = smallp.tile([128, NCAND], BF16, tag="candbf")
                    nc.scalar.copy(candbf[:, :], cand[:, :])
                    return (j, cand, lidx, Om, candbf)

                def compact_stage(state):
                    # prefix ranks on PE + compaction; deferred one tile so
                    # these PE ops never sit ahead of the next tile's d2
                    # matmuls in PE's in-order queue.
                    j, cand, lidx, Om, candbf = state
                    psT0 = psbf.tile([128, 128], BF16, tag="psbf")
                    nc.tensor.transpose(psT0[:, :], Om[:, :], identbf[:, :])
                    ot0 = smallp.tile([128, 128], BF16, tag="ot0")
                    nc.scalar.copy(ot0[:, :], psT0[:, :])
                    psP0 = psatt.tile([128, 128], F32, tag="psatt")
                    nc.tensor.matmul(psP0[:, :], lhsT=ltbf[:, :],
                                     rhs=ot0[:, :], start=True, stop=True)
                    pf0 = smallp.tile([128, 128], BF16, tag="pf0")
                    nc.scalar.copy(pf0[:, :], psP0[:, :])
                    psB0 = psbf.tile([128, 128], BF16, tag="psbf")
                    nc.tensor.transpose(psB0[:, :], pf0[:, :],
                                        identbf[:, :])
                    pfx = smallp.tile([128, NCAND], BF16, tag="pfx")
                    nc.scalar.copy(pfx[:, :], psB0[:, :])
                    om1 = smallp.tile([128, NCAND], BF16, tag="om1")
                    nc.vector.tensor_scalar(om1[:, :], Om[:, :], -1.0, None,
                                            op0=ALU.add)
                    nc.vector.tensor_tensor(pfx[:, :], pfx[:, :], Om[:, :],
                                            op=ALU.mult)
                    sidx = smallp.tile([128, NCAND], I16, tag="sidx")
                    nc.vector.tensor_tensor(sidx[:, :], pfx[:, :], om1[:, :],
                                            op=ALU.add)
                    gidx = smallp.tile([128, 32], I16, tag="gidx")
                    nc.gpsimd.local_scatter(gidx[:, :], lidx[:, :], sidx[:, :],
                                            channels=128, num_elems=32,
                                            num_idxs=NCAND)
                    dval = smallp.tile([128, 32], BF16, tag="dval")
                    nc.gpsimd.local_scatter(dval[:, :], candbf[:, :], sidx[:, :],
                                            channels=128, num_elems=32,
                                            num_idxs=NCAND)
                    dsl = dis[:, j * K:(j + 1) * K]
                    nc.vector.tensor_scalar(dsl, dval[:, 0:K], -1.0,
                                            sqt[:, j:j + 1],
                                            op0=ALU.mult, op1=ALU.add)
                    nc.vector.tensor_scalar_max(dsl, dsl, 1e-12)
                    # wrapped int16 gather-index image (DRAM round-trip on the
                    # Activation DMA queue, off the SP queue)
                    gdram = gdramp.tile([128, K], I16, tag="gdram")
                    nc.scalar.dma_start(gdram[:, :], gidx[:, 0:K])
                    srca = bass.AP(
                        tensor=gdram[:, :].tensor,
                        offset=gdram[:, :].offset,
                        ap=[[K, 16], [1, K], [16 * K, 8]],
                    )
                    ixq = smallp.tile([16, 160], I16, tag="ixq")
                    nc.scalar.dma_start(
                        ixq[:, :].rearrange("q (k g) -> q k g", k=K), srca)
                    ixqf = smallp.tile([16, 160], F32, tag="ixqf")
                    nc.scalar.copy(ixqf[:, :], ixq[:, :])
                    return (j, ixqf)

                def image_stage(state):
                    # deferred a second tile: the gdram->ixq DMA round-trip
                    # (~4us) finishes during the next tile's scans, so this
                    # PE matmul never blocks the following d2 matmuls
                    j, ixqf = state
                    psI = psmarr.tile([128, 512], F32, tag="psmarr")
                    nc.tensor.matmul(psI[:, 0:160], lhsT=repm[:, :],
                                     rhs=ixqf[:, :], start=True, stop=True)
                    nc.scalar.copy(idx16[:, j * 160:(j + 1) * 160],
                                   psI[:, 0:160])

                pend_scan = None
                pend_img = None
                for j in tiles:
                    st = scan_stage(j)
                    if pend_img is not None:
                        image_stage(pend_img)
                        pend_img = None
                    if pend_scan is not None:
                        pend_img = compact_stage(pend_scan)
                    pend_scan = st
                pend_img2 = compact_stage(pend_scan)
                if pend_img is not None:
                    image_stage(pend_img)
                image_stage(pend_img2)
                # batched sqrt for this half's distances (bf16)
                hsl = dis[:, h * HALF * K:(h + 1) * HALF * K]
                nc.scalar.activation(hsl, hsl, AF.Sqrt)

                # ---- gather section: block-0 slabs, softmax, block-0 att ----
                nc.gpsimd.load_library(library_config.mlp)
                for bp in range(HALF // 2):  # 2-tile batches
                    j0 = h * HALF + bp * 2
                    slabr = slabp.tile([128, 4 * K, H], BF16, tag="slab")
                    slab0 = slabr[:, :, :].rearrange(
                        "p (j two) h -> p j (two h)", two=2)
                    nc.gpsimd.dma_gather(
                        out_ap=slab0, in_ap=d_nd0.ap(),
                        idxs_ap=idx16[:, j0 * 160:(j0 + 2) * 160],
                        num_idxs=128 * K * 2, num_idxs_reg=128 * K * 2,
                        elem_size=ND0W, single_packet=False,
                    )
                    # softmax for both tiles, all 6 blocks
                    d2sl = dis[:, j0 * K:(j0 + 2) * K]
                    sraw = slabr[:, :, :]
                    for i in range(NB):
                        # G[idx] payload: raw row 2*slot+1, col i
                        gap = bass.AP(
                            tensor=sraw.tensor,
                            offset=sraw.offset + H + i,
                            ap=[sraw.ap[0], [2 * H, 2 * K]],
                        )
                        sc = smallp.tile([128, 2 * K], BF16, tag="sc")
                        nc.vector.scalar_tensor_tensor(
                            sc[:, :], d2sl, wc0c[:, i:i + 1], gap,
                            op0=ALU.mult, op1=ALU.add)
                        esc = smallp.tile([128, 2 * K], BF16, tag="esc")
                        nc.scalar.activation(esc[:, :], sc[:, :], AF.Exp)
                        esc3 = esc[:, :].rearrange("p (j k) -> p j k", k=K)
                        den = smallp.tile([128, 2], F32, tag="den")
                        nc.vector.tensor_reduce(den[:, :], esc3, axis=AX.X,
                                                op=ALU.add)
                        rden = smallp.tile([128, 2], F32, tag="rden")
                        nc.vector.reciprocal(rden[:, :], den[:, :])
                        rb = rden[:, :]
                        rbc = bass.AP(tensor=rb.tensor, offset=rb.offset,
                                      ap=[rb.ap[0], [1, 2], [0, K]])
                        w20s = w20all[:, i * NT * K + j0 * K:
                                      i * NT * K + (j0 + 2) * K].rearrange(
                            "p (j k) -> p j k", k=K)
                        nc.vector.tensor_tensor(w20s, esc3, rbc, op=ALU.mult)
                    # block-0 attention for both tiles
                    for jj in range(2):
                        j = j0 + jj
                        attention(
                            0, j,
                            lambda k, _jj=jj: slabr[:, 2 * (_jj * K + k), :],
                            relu0[:, j * 128:(j + 1) * 128],
                            att0T[:, j * 128:(j + 1) * 128],
                        )

                    if h == 1 and bp % 2 == 1:
                        b0_group(4 + bp // 2)

                if h == 0:
                    for g in range(4):
                        b0_group(g)

            def gather128(out_ap, in_ap, idxs_ap, num_idxs):
                """fp8 gather: 128B payload per descriptor at 256B row stride.
                The bass wrapper requires elem_size_bytes % 256 == 0; the
                descriptor format only constrains the STRIDE to 256B units,
                so emit InstDMAGatherAnt directly."""
                gp = nc.gpsimd
                _in = gp.lower_ap_dma(in_ap, for_custom_bir_dma=True)
                _idx = gp.lower_ap(idxs_ap)
                _out = gp.lower_ap(out_ap)
                return gp.add_instruction(mybir.InstDMAGatherAnt(
                    name=gp.bass.get_next_instruction_name(),
                    ins=[*_in, _idx,
                         gp.lower_val_access(gp.to_reg(num_idxs))],
                    outs=[_out],
                    transpose=False,
                    num_idxs=num_idxs,
                    elem_size=H,
                    stride_bytes_256=1,
                    gen_mode=0,
                    single_packet=False,
                    queue_num=0,
                    sbuf_tokens_per_rank=0,
                    sbuf_free_dim_per_rank=0,
                    sbuf_free_dim_pad_per_rank=0,
                    sbuf_byte_offset=0,
                ))


            # ================= blocks 1..5 =================
            stkA.close()
            slabp8 = stk.enter_context(tc.tile_pool(name="slab8", bufs=4))
            dstk2p = stk.enter_context(tc.tile_pool(name="dstk2", bufs=2))
            gbufB = stk.enter_context(tc.tile_pool(name="gbufB", bufs=3))
            psattB = stk.enter_context(tc.tile_pool(name="psattB", bufs=2, space="PSUM"))
            psbigB = stk.enter_context(tc.tile_pool(name="psbigB", bufs=2, space="PSUM"))
            for i in range(1, NB):
                nin = netB if i % 2 == 1 else netA
                nout = netA if i % 2 == 1 else netB
                if i < NB - 1:
                    netdram_next = dramp.tile([T, ND8W], F8, tag="netdram")
                else:
                    netdram_next = None
                def att_stage(g):
                    slab = slabp8.tile([128, 4 * K, H], F8, tag="slab8")
                    gather128(slab[:, :, :], netdram[:, 0:H],
                              idx16[:, (4 * g) * 160:(4 * g + 4) * 160],
                              128 * K * 4)
                    attT4 = gbufB.tile([128, 512], BF16, tag="attT4B")
                    reluA4 = gbufB.tile([128, 512], BF16, tag="reluA4B")
                    for jp in range(2):
                        # one DVE op builds BOTH tiles' diag stacks (halves
                        # the DVE->PE handoff count in the unit chain)
                        ds2 = dstk2p.tile([128, 2 * 128 * K], BF16, tag="ds2")
                        ds24 = ds2[:, :].rearrange(
                            "p (jj s k) -> p jj s k", jj=2, k=K)
                        j0 = 4 * g + 2 * jp
                        wbase = w20all[:, (i * NT + j0) * K:
                                       (i * NT + j0 + 2) * K]
                        wb = bass.AP(
                            tensor=wbase.tensor, offset=wbase.offset,
                            ap=[wbase.ap[0], [K, 2], [0, 128], [1, K]])
                        idr4 = bass.AP(tensor=idrep3.tensor,
                                       offset=idrep3.offset,
                                       ap=[idrep3.ap[0], [0, 2],
                                           idrep3.ap[1], idrep3.ap[2]])
                        nc.vector.tensor_tensor(ds24, idr4, wb, op=ALU.mult)
                        for jj2 in range(2):
                            jj = 2 * jp + jj2
                            j = 4 * g + jj
                            attention(
                                i, j,
                                lambda k, _jj=jj: slab[:, _jj * K + k, :],
                                reluA4[:, jj * 128:(jj + 1) * 128],
                                attT4[:, jj * 128:(jj + 1) * 128],
                                psApool=psattB,
                                dstack3=ds24[:, jj2, :, :],
                            )
                    return (g, attT4, reluA4)

                def resnet_stage(state):
                    # deferred one group so PE's in-order queue runs the next
                    # group's attention matmuls instead of stalling on ACT's
                    # hrelu (same fix that unblocked the phase-A scans)
                    g, attT4, reluA4 = state
                    nsl = slice(g * 512, (g + 1) * 512)
                    reluNt = gbufp.tile([128, 512], BF16, tag="reluN")
                    nc.scalar.activation(reluNt[:, :], nin[:, nsl], AF.Relu)
                    reluN = reluNt[:, :]
                    netbft = gbufp.tile([128, 512], BF16, tag="netbf")
                    nc.scalar.copy(netbft[:, :], nin[:, nsl])
                    netbf = netbft[:, :]
                    ps1 = psbig.tile([128, 512], F32, tag="psbig")
                    nc.tensor.matmul(ps1[:, :], lhsT=w0a[:, i, :],
                                     rhs=reluN, start=True, stop=False)
                    nc.tensor.matmul(ps1[:, :], lhsT=w0b[:, i, :],
                                     rhs=reluA4[:, :], start=False, stop=True)
                    hrelu = gbufp.tile([128, 512], BF16, tag="hrelu")
                    nc.scalar.activation(hrelu[:, :], ps1[:, :], AF.Relu,
                                         bias=b0col[:, i:i + 1], scale=1.0)
                    ps2 = psbigB.tile([128, 512], F32, tag="psbigB")
                    nc.tensor.matmul(ps2[:, :], lhsT=w1[:, i, :],
                                     rhs=hrelu[:, :], start=True, stop=False)
                    nc.tensor.matmul(ps2[:, :], lhsT=wsa[:, i, :],
                                     rhs=netbf, start=False, stop=False)
                    nc.tensor.matmul(ps2[:, :], lhsT=wsb[:, i, :],
                                     rhs=attT4[:, :], start=False, stop=False)
                    nc.tensor.matmul(ps2[:, :],
                                     lhsT=browbf[:, i * 128:(i + 1) * 128],
                                     rhs=browbf[:, 13 * 128:13 * 128 + 512],
                                     start=False, stop=True)
                    nc.vector.tensor_tensor(nout[:, nsl], ps2[:, :],
                                            nin[:, nsl], op=ALU.add)
                    if i == NB - 1:
                        # fused final projection for this group
                        n6bf = gbufp.tile([128, 512], BF16, tag="nob")
                        nc.scalar.copy(n6bf[:, :], nout[:, nsl])
                        psF = psbig.tile([128, 512], F32, tag="psbig")
                        nc.tensor.matmul(psF[:, :], lhsT=wcfbf[:, :],
                                         rhs=n6bf[:, :], start=True, stop=False)
                        nc.tensor.matmul(psF[:, :],
                                         lhsT=browbf[:, 12 * 128:13 * 128],
                                         rhs=browbf[:, 13 * 128:13 * 128 + 512],
                                         start=False, stop=True)
                        og = outbp.tile([128, 512], F32, tag="og")
                        nc.scalar.copy(og[:, :], psF[:, :])
                        nc.sync.dma_start(d_out.ap()[:, nsl], og[:, :])
                    if netdram_next is not None:
                        nob = gbufp.tile([128, 512], BF16, tag="nob")
                        nc.scalar.copy(nob[:, :], nout[:, nsl])
                        nrow4 = sbw.tile([128, 512], BF16, tag="nrow4")
                        for jj in range(4):
                            nc.sync.dma_start_transpose(
                                nrow4[:, jj * 128:(jj + 1) * 128],
                                nob[:, jj * 128:(jj + 1) * 128])
                        nrowf8 = sbw.tile([128, 512], F8, tag="nrowf8")
                        nc.scalar.copy(nrowf8[:, :], nrow4[:, :])
                        nc.sync.dma_start(
                            netdram_next[g * 512:(g + 1) * 512, 0:H].rearrange(
                                "(jj t) h -> t jj h", jj=4),
                            nrowf8[:, :].rearrange("t (jj h) -> t jj h", h=H))

                pend_g = None
                for g in range(8):
                    stg = att_stage(g)
                    if pend_g is not None:
                        resnet_stage(pend_g)
                    pend_g = stg
                resnet_stage(pend_g)
                netdram = netdram_next

    nc.compile()
    return nc


def make_inputs(p_all, weights):
    """Build the per-core input maps. p_all: [B, T, D] f32."""
    w = weights
    bf = ml_dtypes.bfloat16
    shared = {}
    wc = np.asarray(w["att_Wc"][:, :, 0], np.float32)  # [NB, 7]
    shared["wpos4"] = np.concatenate(
        [w["W_pos"], w["b_pos"][None, :]], axis=0
    ).astype(np.float32)  # [4, H]
    shared["wc0c"] = np.broadcast_to(wc[:, 0][None, :], (128, NB)).astype(
        np.float32).copy()
    shared["b0col"] = np.ascontiguousarray(
        np.asarray(w["blk_b0"], np.float32).T)  # [H, NB]
    shared["w0a"] = np.ascontiguousarray(w["blk_W0"][:, :H, :]).astype(bf)
    shared["w0b"] = np.ascontiguousarray(w["blk_W0"][:, H:, :]).astype(bf)
    shared["w1"] = np.ascontiguousarray(w["blk_W1"]).astype(bf)
    shared["wsa"] = np.ascontiguousarray(w["blk_Ws"][:, :H, :]).astype(bf)
    shared["wsb"] = np.ascontiguousarray(w["blk_Ws"][:, H:, :]).astype(bf)
    shared["wo"] = np.ascontiguousarray(w["att_Wo"]).astype(bf)
    shared["wcfbf"] = np.ascontiguousarray(w["W_c"]).astype(bf)
    shared["identbf"] = np.eye(128, dtype=np.float32).astype(bf)
    idr = np.zeros((128, 128, K), np.float32)
    for t in range(128):
        idr[t, t, :] = 1.0
    shared["identrep"] = idr.reshape(128, 128 * K).astype(bf)
    brow = np.zeros((1, 18 * 128), np.float32)
    for i in range(NB):
        brow[0, i * 128:(i + 1) * 128] = w["blk_b1"][i]
        brow[0, (6 + i) * 128:(7 + i) * 128] = w["att_bo"][i]
    brow[0, 12 * 128:13 * 128] = w["b_c"]
    brow[0, 13 * 128:17 * 128] = 1.0
    shared["browbf"] = brow.astype(bf)
    shared["ltbf"] = np.triu(np.ones((128, 128), np.float32), 1).T.astype(bf)
    co = np.zeros((128, NCAND), np.uint16)
    co[:, :] = (np.arange(NCAND) // 8 * CHW)[None, :]
    shared["chunkoff"] = co
    rm = np.zeros((16, 128), np.float32)
    for p in range(128):
        rm[p % 16, p] = 1.0
    shared["repmat"] = rm

    wcg = wc[:, 1:4]  # [NB, 3]
    wpos = np.asarray(w["W_pos"], np.float32)
    bpos = np.asarray(w["b_pos"], np.float32)

    in_maps = []
    for c in range(B):
        p = np.asarray(p_all[c], np.float32)  # [T, D]
        sq = (p * p).sum(-1)  # [T]
        m = dict(shared)
        lhsT4 = np.ones((4, T), np.float32)
        lhsT4[0:3, :] = p.T
        m["lhsT4"] = lhsT4
        rhs4 = np.empty((4, T), np.float32)
        rhs4[0:3, :] = 2.0 * p.T
        rhs4[3, :] = -sq
        m["rhs4"] = rhs4
        m["sqt"] = np.ascontiguousarray(sq.reshape(NT, 128).T)  # [128, NT]
        nd0 = np.zeros((T, ND0W), np.float32)
        nd0[:, 0:H] = p @ wpos + bpos          # net0 rows
        nd0[:, H:H + NB] = p @ wcg.T           # G[s, i]
        m["nd0"] = nd0.astype(bf)
        in_maps.append(m)
    return in_maps


_PROGRAM = None


def kernel(**inputs):
    global _PROGRAM
    p_all = np.asarray(inputs["p"], np.float32)
    assert p_all.shape == (B, T, D)
    in_maps = make_inputs(p_all, {k: np.asarray(v) for k, v in inputs.items()})
    if _PROGRAM is None:
        _PROGRAM = build_program()
    from concourse import bass_utils
    res = bass_utils.run_bass_kernel_spmd(
        _PROGRAM, in_maps, core_ids=list(range(B))
    )
    out = np.stack([r["outp"].T for r in res.results], axis=0)  # [B, T, CDIM]
    return np.ascontiguousarray(out).astype(np.float32)


if __name__ == "__main__":
    # smoke: build only
    nc = build_program()
    print("built ok")
